# revision 28
# baseline (speedup 1.0000x reference)
"""Two-layer GAT (DGL GATConv style) on 8 Trainium2 NeuronCores via Bass/Tile.

Strategy (dst-partitioned graph parallel):
  - Nodes are split into 8 contiguous dst ranges; each core owns the edges
    whose dst falls in its range (host sorts/pads edge lists).
  - Per layer: each core projects its own node slice (h, el, er fall out of
    one matmul against an augmented weight matrix), packs 288B table rows
    [h bf16 x128 | el f32 x4 | er f32 x4], AllGathers the full node table,
    then per 128-edge tile gathers h_aug[src] rows with indirect DMA,
    computes edge softmax (shift-invariant, so segment-max is skipped) and
    aggregates with one bf16 matmul per tile against an on-chip-built
    one-hot segment matrix; appending the softmax weights as extra rhs
    columns yields the per-dst normalizers in the same matmul.
"""
import sys

sys.path.insert(0, "/opt/trn_rl_repo")

import math
from contextlib import ExitStack

import ml_dtypes
import numpy as np

import concourse.bass as bass
import concourse.mybir as mybir
import concourse.tile as tile
from concourse._compat import with_exitstack
from concourse.masks import make_identity

NEG = 0.2
ROW = 144  # bf16 elems per table row: h(128) + el f32(as 8) + er f32(as 8)
F = 128
H = 4
OUT = 32

bf16 = mybir.dt.bfloat16
f32 = mybir.dt.float32
i32 = mybir.dt.int32


class Cfg:
    def __init__(self, n, e, ncores, sup=32):
        self.N = n
        self.E = e
        self.NC = ncores
        self.NPC = n // ncores              # owned dsts per core
        self.NB = math.ceil(self.NPC / 128)  # dst blocks per core
        self.ROWS = self.NB * 128            # padded slice rows per core
        self.TROWS = self.ROWS * ncores
        self.SUP = sup                       # tiles per elementwise supertile
        self.TAIL = self.NPC - (self.NB - 1) * 128  # valid rows in last block
        assert self.TAIL < 128, "need at least one pad row for the sentinel"
        self.SENT = self.ROWS - 1            # sentinel table row (core 0 tail)


def prep_inputs(cfg, src, dst):
    """Host-side: per-core edge layout. Returns per-core arrays + inverse perms.

    Per core:
      offs_src [128, T] i32 : table row of edge's src (sentinel for pads)
      offs_dst [128, T] i32 : table row of edge's dst
      dloc     [128, T] bf16: dst slot within its 128-dst block
    Shared: blk_of[t], k_of[t], Tb[b] block structure (identical across cores).
    """
    src = np.asarray(src).astype(np.int64)
    dst = np.asarray(dst).astype(np.int64)
    NPC, NB, ROWS, NC = cfg.NPC, cfg.NB, cfg.ROWS, cfg.NC

    per_core = []
    perms = []
    cnts = np.zeros((NC, NB), np.int64)
    for c in range(NC):
        eid = np.nonzero(dst // NPC == c)[0]
        d0 = (dst[eid] - c * NPC).astype(np.int64)
        # degree-balance: relabel dsts so block loads are even
        counts = np.bincount(d0, minlength=NPC)
        order = np.argsort(-counts, kind="stable")
        perm = np.empty(NPC, np.int64)
        ranks = np.arange(NPC)
        perm[order] = (ranks % NB) * 128 + ranks // NB
        rd = perm[d0]
        eorder = np.argsort(rd, kind="stable")
        esrc, erd, ed0 = src[eid][eorder], rd[eorder], d0[eorder]
        bid = erd // 128
        cnts[c] = np.bincount(bid, minlength=NB)
        per_core.append((esrc, erd, ed0, bid))
        perms.append(perm)

    Tb = np.maximum(1, np.ceil(cnts.max(axis=0) / 128).astype(np.int64))
    T = int(Tb.sum())
    t0 = np.concatenate([[0], np.cumsum(Tb)])[:-1]
    blk_of = np.concatenate([np.full(Tb[b], b, np.int64) for b in range(NB)])
    k_of = np.concatenate([np.arange(Tb[b]) for b in range(NB)])

    # table-1 rows are in original local order; table-2 rows (built from the
    # layer-1 output) are in relabeled slot order — so layer 2 gets its own
    # gather index arrays mapped through the owning core's permutation.
    permcat = np.concatenate(perms)  # [NC*NPC] local slot of (core, localid)

    ins = []
    for c in range(NC):
        esrc, erd, ed0, bid = per_core[c]
        osrc = np.full((128, T), cfg.SENT, np.int32)
        odst = np.full((128, T), cfg.SENT, np.int32)
        osrc2 = np.full((128, T), cfg.SENT, np.int32)
        odst2 = np.full((128, T), cfg.SENT, np.int32)
        dl = np.zeros((128, T), np.float32)
        # edges are sorted by rd hence grouped by block
        boff = np.concatenate([[0], np.cumsum(np.bincount(bid, minlength=NB))])
        score = esrc // NPC
        srow = score * ROWS + esrc % NPC          # layer-1 src table row
        drow = c * ROWS + ed0                     # layer-1 er row (orig order)
        srow2 = score * ROWS + permcat[esrc]      # layer-2 src row (slot order)
        drow2 = c * ROWS + erd                    # layer-2 er row (slot order)
        for b in range(NB):
            j = np.arange(boff[b], boff[b + 1]) - boff[b]
            cols = t0[b] + j // 128
            parts = j % 128
            sl = slice(boff[b], boff[b + 1])
            osrc[parts, cols] = srow[sl]
            odst[parts, cols] = drow[sl]
            osrc2[parts, cols] = srow2[sl]
            odst2[parts, cols] = drow2[sl]
            dl[parts, cols] = (erd[sl] % 128).astype(np.float32)
        ins.append(
            dict(
                osrc=osrc,
                odst=odst,
                osrc2=osrc2,
                odst2=odst2,
                dloc=dl.astype(ml_dtypes.bfloat16),
            )
        )
    return ins, perms, Tb.tolist(), T, blk_of.tolist(), k_of.tolist()


def aug_weights(W, al, ar):
    """[128, 136] f32: [W | W@al_h | W@ar_h]."""
    Wa = np.zeros((F, 136), np.float32)
    Wa[:, :F] = W
    for h in range(H):
        Wa[:, F + h] = W[:, h * OUT:(h + 1) * OUT] @ al[h]
        Wa[:, F + H + h] = W[:, h * OUT:(h + 1) * OUT] @ ar[h]
    return Wa


@with_exitstack
def build_kernel(ctx: ExitStack, tc: tile.TileContext, cfg, Tb, T, blk_of, k_of,
                 dbg=False):
    nc = tc.nc
    NB, ROWS, TROWS, SUP, TAIL = cfg.NB, cfg.ROWS, cfg.TROWS, cfg.SUP, cfg.TAIL
    internal = dict(kind="ExternalOutput") if dbg else {}

    # --- I/O ---
    featT = nc.dram_tensor("featT", [F, ROWS], f32, kind="ExternalInput")
    w1 = nc.dram_tensor("w1aug", [F, 136], f32, kind="ExternalInput")
    w2 = nc.dram_tensor("w2aug", [F, 136], f32, kind="ExternalInput")
    osrc = nc.dram_tensor("osrc", [128, T], i32, kind="ExternalInput")
    odst = nc.dram_tensor("odst", [128, T], i32, kind="ExternalInput")
    osrc2 = nc.dram_tensor("osrc2", [128, T], i32, kind="ExternalInput")
    odst2 = nc.dram_tensor("odst2", [128, T], i32, kind="ExternalInput")
    dlocd = nc.dram_tensor("dloc", [128, T], bf16, kind="ExternalInput")
    sentel = nc.dram_tensor("sentel", [1, 16], bf16, kind="ExternalInput")
    iotad = nc.dram_tensor("iota", [128, 128], bf16, kind="ExternalInput")
    b1d = nc.dram_tensor("b1rep", [128, F], f32, kind="ExternalInput")
    b2d = nc.dram_tensor("b2rep", [128, OUT], f32, kind="ExternalInput")
    out_ext = nc.dram_tensor("out", [ROWS, OUT], f32, kind="ExternalOutput")

    slice1 = nc.dram_tensor("slice1", [ROWS, ROW], bf16)
    slice2 = nc.dram_tensor("slice2", [ROWS, ROW], bf16)
    table1 = nc.dram_tensor("table1", [TROWS, ROW], bf16, addr_space="Shared")
    table2 = nc.dram_tensor("table2", [TROWS, ROW], bf16, addr_space="Shared")
    h1r = nc.dram_tensor("h1r", [ROWS, F], f32, **internal)

    core_ids = list(range(cfg.NC))

    consts = ctx.enter_context(tc.tile_pool(name="consts", bufs=1))
    featp = ctx.enter_context(tc.tile_pool(name="featp", bufs=1))
    offp = ctx.enter_context(tc.tile_pool(name="offp", bufs=1))
    projp = ctx.enter_context(tc.tile_pool(name="projp", bufs=3))
    projps = ctx.enter_context(tc.tile_pool(name="projps", bufs=2, space="PSUM"))
    gp = ctx.enter_context(tc.tile_pool(name="gp", bufs=2))
    ep = ctx.enter_context(tc.tile_pool(name="ep", bufs=2))
    accp = ctx.enter_context(tc.tile_pool(name="accp", bufs=3, space="PSUM"))
    outp = ctx.enter_context(tc.tile_pool(name="outp", bufs=2))

    w1_sb = consts.tile([F, 136], f32)
    w2_sb = consts.tile([F, 136], f32)
    iota_sb = consts.tile([128, 128], bf16)
    b1_sb = consts.tile([128, F], f32)
    b2_sb = consts.tile([128, OUT], f32)
    ident = consts.tile([128, 128], f32)
    nc.sync.dma_start(w1_sb[:], w1[:])
    nc.sync.dma_start(w2_sb[:], w2[:])
    nc.sync.dma_start(iota_sb[:], iotad[:])
    nc.sync.dma_start(b1_sb[:], b1d[:])
    nc.sync.dma_start(b2_sb[:], b2d[:])
    make_identity(nc, ident[:])

    featT_sb = featp.tile([F, ROWS], f32)
    nc.sync.dma_start(featT_sb[:], featT[:])
    osrc_sb = offp.tile([128, T], i32)
    odst_sb = offp.tile([128, T], i32)
    osrc2_sb = offp.tile([128, T], i32)
    odst2_sb = offp.tile([128, T], i32)
    dloc_sb = offp.tile([128, T], bf16)
    nc.sync.dma_start(osrc_sb[:], osrc[:])
    nc.sync.dma_start(odst_sb[:], odst[:])
    nc.sync.dma_start(osrc2_sb[:], osrc2[:])
    nc.sync.dma_start(odst2_sb[:], odst2[:])
    nc.sync.dma_start(dloc_sb[:], dlocd[:])

    def emit_row_tile(ph, nt, slice_dram, sentinel_tail):
        """psum [128,136] f32 -> packed bf16 row tile -> DRAM slice.

        sentinel_tail (layer 1, original row order): rows >= TAIL of the last
        tile are sentinels (h=0, el=-1e9 so exp->0). Partition starts must be
        32-aligned, so memset an aligned superset first and overwrite the
        valid rows with the real copies.
        """
        row_t = projp.tile([128, ROW], bf16, tag="rowt")
        rv = row_t[:, :].bitcast(f32)  # [128, 72] f32 view
        if sentinel_tail and nt == NB - 1:
            astart = (TAIL // 32) * 32
            nc.vector.memset(row_t[astart:128, :], 0.0)
            nc.vector.memset(rv[astart:128, 64:68], -1e9)
            nc.vector.tensor_copy(row_t[0:TAIL, 0:F], ph[0:TAIL, 0:F])
            nc.vector.tensor_copy(rv[0:TAIL, 64:72], ph[0:TAIL, F:136])
        else:
            nc.vector.tensor_copy(row_t[:, 0:F], ph[:, 0:F])
            nc.vector.tensor_copy(rv[:, 64:72], ph[:, F:136])
        nc.sync.dma_start(slice_dram[nt * 128:(nt + 1) * 128, :], row_t[:])

    # ---------- Phase P1: project own slice with W1_aug ----------
    for nt in range(NB):
        ph = projps.tile([128, 136], f32, tag="ph")
        nc.tensor.matmul(out=ph[:], lhsT=featT_sb[:, nt * 128:(nt + 1) * 128],
                         rhs=w1_sb[:], start=True, stop=True)
        emit_row_tile(ph, nt, slice1, sentinel_tail=True)

    tc.strict_bb_all_engine_barrier()
    nc.gpsimd.collective_compute(
        "AllGather", mybir.AluOpType.bypass, replica_groups=[core_ids],
        ins=[slice1[:]], outs=[table1[:]])
    tc.strict_bb_all_engine_barrier()

    # ---------- Edge phase ----------
    def edge_phase(table, layer, os_sb, od_sb):
        acc_box = [None]
        for t0 in range(0, T, SUP):
            K = min(SUP, T - t0)
            # HW indirect DMA consumes ONE offset per partition (multi-column
            # offset APs silently gather consecutive rows) -> one call per tile.
            g = gp.tile([128, SUP * ROW], bf16, tag="g")
            erg = gp.tile([128, SUP * 8], bf16, tag="erg")
            for k in range(K):
                t = t0 + k
                nc.gpsimd.indirect_dma_start(
                    out=g[:, k * ROW:(k + 1) * ROW], out_offset=None, in_=table[:],
                    in_offset=bass.IndirectOffsetOnAxis(ap=os_sb[:, t:t + 1], axis=0))
                nc.gpsimd.indirect_dma_start(
                    out=erg[:, k * 8:(k + 1) * 8], out_offset=None, in_=table[:],
                    in_offset=bass.IndirectOffsetOnAxis(ap=od_sb[:, t:t + 1], axis=0),
                    element_offset=136)

            g32 = g[:, :].bitcast(f32)    # [128, SUP*72]
            er32 = erg[:, :K * 8].bitcast(f32)  # [128, K*4]
            logit = ep.tile([128, SUP * 4], f32, tag="logit")
            el_ap = bass.AP(tensor=g32.tensor, offset=g32.offset + 64,
                            ap=[g32.ap[0], [72, K], [1, 4]])
            nc.vector.tensor_tensor(out=logit[:, :K * 4], in0=el_ap, in1=er32,
                                    op=mybir.AluOpType.add)
            lrl = ep.tile([128, SUP * 4], f32, tag="lrl")
            nc.vector.tensor_scalar_mul(lrl[:, :K * 4], logit[:, :K * 4], NEG)
            nc.vector.tensor_tensor(out=lrl[:, :K * 4], in0=logit[:, :K * 4],
                                    in1=lrl[:, :K * 4], op=mybir.AluOpType.max)
            # clamp: sentinel logits are ~-2e8, outside the HW exp table range
            nc.vector.tensor_scalar_max(lrl[:, :K * 4], lrl[:, :K * 4], -80.0)
            p_t = ep.tile([128, SUP * 4], bf16, tag="p")
            nc.scalar.activation(p_t[:, :K * 4], lrl[:, :K * 4],
                                 mybir.ActivationFunctionType.Exp)

            s0 = ep.tile([128, SUP * 128], bf16, tag="s0")
            io = iota_sb[:, :]
            dl = dloc_sb[:, t0:t0 + K]
            iota_ap = bass.AP(tensor=io.tensor, offset=io.offset,
                              ap=[io.ap[0], [0, K], [1, 128]])
            dloc_ap = bass.AP(tensor=dl.tensor, offset=dl.offset,
                              ap=[dl.ap[0], [1, K], [0, 128]])
            nc.vector.tensor_tensor(out=s0[:, :K * 128], in0=iota_ap, in1=dloc_ap,
                                    op=mybir.AluOpType.is_equal)

            rhs = ep.tile([128, SUP * 132], bf16, tag="rhs")
            gb, pb, rb = g[:, :], p_t[:, :], rhs[:, :]
            for hh in range(H):
                in0 = bass.AP(tensor=gb.tensor, offset=gb.offset + hh * OUT,
                              ap=[gb.ap[0], [ROW, K], [1, OUT]])
                in1 = bass.AP(tensor=pb.tensor, offset=pb.offset + hh,
                              ap=[pb.ap[0], [4, K], [0, OUT]])
                o = bass.AP(tensor=rb.tensor, offset=rb.offset + hh * OUT,
                            ap=[rb.ap[0], [132, K], [1, OUT]])
                nc.vector.tensor_tensor(out=o, in0=in0, in1=in1,
                                        op=mybir.AluOpType.mult)
            pco = bass.AP(tensor=rb.tensor, offset=rb.offset + 128,
                          ap=[rb.ap[0], [132, K], [1, 4]])
            pci = bass.AP(tensor=pb.tensor, offset=pb.offset,
                          ap=[pb.ap[0], [4, K], [1, 4]])
            nc.vector.tensor_copy(out=pco, in_=pci)

            for k in range(K):
                t = t0 + k
                b = blk_of[t]
                if k_of[t] == 0:
                    acc_box[0] = accp.tile([128, 132], f32, tag="acc", name="acc")
                acc = acc_box[0]
                nc.tensor.matmul(
                    out=acc[:], lhsT=s0[:, k * 128:(k + 1) * 128],
                    rhs=rhs[:, k * 132:(k + 1) * 132],
                    start=(k_of[t] == 0), stop=(k_of[t] == Tb[b] - 1))
                if k_of[t] == Tb[b] - 1:
                    s_eps = outp.tile([128, 4], f32, tag="seps")
                    nc.vector.tensor_scalar_add(s_eps[:], acc[:, 128:132], 1e-30)
                    inv = outp.tile([128, 4], f32, tag="inv")
                    nc.vector.reciprocal(inv[:], s_eps[:])
                    iv = inv[:, :]
                    iv_ap = bass.AP(tensor=iv.tensor, offset=iv.offset,
                                    ap=[iv.ap[0], [1, 4], [0, OUT]])
                    if layer == 1:
                        of = outp.tile([128, F], f32, tag="of")
                        nc.vector.tensor_tensor(out=of[:], in0=acc[:, :F],
                                                in1=iv_ap, op=mybir.AluOpType.mult)
                        nc.vector.tensor_tensor(out=of[:], in0=of[:], in1=b1_sb[:],
                                                op=mybir.AluOpType.add)
                        nc.vector.tensor_scalar_max(of[:], of[:], 0.0)
                        nc.sync.dma_start(h1r[b * 128:(b + 1) * 128, :], of[:])
                    else:
                        nc.vector.tensor_scalar_mul(inv[:], inv[:], 0.25)
                        tmp = outp.tile([128, F], f32, tag="tmp2")
                        nc.vector.tensor_tensor(out=tmp[:], in0=acc[:, :F],
                                                in1=iv_ap, op=mybir.AluOpType.mult)
                        om = outp.tile([128, OUT], f32, tag="om")
                        tv = tmp[:, :]
                        tv_ap = bass.AP(tensor=tv.tensor, offset=tv.offset,
                                        ap=[tv.ap[0], [1, OUT], [OUT, 4]])
                        nc.vector.tensor_reduce(out=om[:], in_=tv_ap,
                                                axis=mybir.AxisListType.X,
                                                op=mybir.AluOpType.add)
                        nc.vector.tensor_tensor(out=om[:], in0=om[:], in1=b2_sb[:],
                                                op=mybir.AluOpType.add)
                        nc.sync.dma_start(out_ext[b * 128:(b + 1) * 128, :], om[:])

    edge_phase(table1, 1, osrc_sb, odst_sb)
    tc.strict_bb_all_engine_barrier()

    # ---------- Phase P2: project relu(h1) slice with W2_aug ----------
    for nt in range(NB):
        h1_t = projp.tile([128, F], f32, tag="h1t")
        nc.sync.dma_start(h1_t[:], h1r[nt * 128:(nt + 1) * 128, :])
        pt = projps.tile([128, 128], f32, tag="ptr")
        nc.tensor.transpose(out=pt[:], in_=h1_t[:], identity=ident[:])
        h1T = projp.tile([128, 128], f32, tag="h1T")
        nc.vector.tensor_copy(h1T[:], pt[:])
        ph = projps.tile([128, 136], f32, tag="ph")
        nc.tensor.matmul(out=ph[:], lhsT=h1T[:], rhs=w2_sb[:], start=True, stop=True)
        emit_row_tile(ph, nt, slice2, sentinel_tail=False)

    # table-2 is in relabeled slot order; the sentinel slot (last row) must
    # still read as "no edge": patch its el to -1e9 (DRAM->DRAM DMA crashes
    # neuronxcc, so bounce through SBUF).
    sent_sb = consts.tile([1, 16], bf16)
    nc.sync.dma_start(sent_sb[:], sentel[:])
    tc.strict_bb_all_engine_barrier()
    nc.sync.dma_start(slice2[ROWS - 1:ROWS, 128:144], sent_sb[:])
    tc.strict_bb_all_engine_barrier()
    nc.gpsimd.collective_compute(
        "AllGather", mybir.AluOpType.bypass, replica_groups=[core_ids],
        ins=[slice2[:]], outs=[table2[:]])
    tc.strict_bb_all_engine_barrier()

    edge_phase(table2, 2, osrc2_sb, odst2_sb)

    if dbg:
        dbg1 = nc.dram_tensor("dbg1", [ROWS, ROW], bf16, kind="ExternalOutput")
        dbg2 = nc.dram_tensor("dbg2", [ROWS, ROW], bf16, kind="ExternalOutput")
        tc.strict_bb_all_engine_barrier()
        for nt in range(NB):
            for srcd, dstd in ((slice1, dbg1), (slice2, dbg2)):
                bt = projp.tile([128, ROW], bf16, tag="dbgb", name="dbgb")
                nc.sync.dma_start(bt[:], srcd[nt * 128:(nt + 1) * 128, :])
                nc.sync.dma_start(dstd[nt * 128:(nt + 1) * 128, :], bt[:])


def build_nc(cfg, Tb, T, blk_of, k_of, compile=True, dbg=False):
    from concourse import bacc

    nc = bacc.Bacc("TRN2", target_bir_lowering=False)
    with tile.TileContext(nc) as tc:
        build_kernel(tc, cfg, Tb, T, blk_of, k_of, dbg=dbg)
    if compile:
        nc.compile()
    return nc


def make_in_maps(cfg, per_core_edges, feat, W1, al1, ar1, b1, W2, al2, ar2, b2):
    w1a = aug_weights(np.asarray(W1, np.float32), np.asarray(al1, np.float32),
                      np.asarray(ar1, np.float32))
    w2a = aug_weights(np.asarray(W2, np.float32), np.asarray(al2, np.float32),
                      np.asarray(ar2, np.float32))
    iota = np.broadcast_to(np.arange(128, dtype=np.float32), (128, 128))
    iota = np.ascontiguousarray(iota.astype(ml_dtypes.bfloat16))
    sentel = np.full((1, 8), -1e9, np.float32).view(np.uint16).reshape(1, 16)
    sentel = sentel.view(ml_dtypes.bfloat16)
    b1r = np.ascontiguousarray(np.broadcast_to(
        np.asarray(b1, np.float32).reshape(1, F), (128, F)))
    b2m = np.asarray(b2, np.float32).reshape(H, OUT).mean(axis=0)
    b2r = np.ascontiguousarray(np.broadcast_to(b2m.reshape(1, OUT), (128, OUT)))
    feat = np.asarray(feat, np.float32)
    in_maps = []
    for c in range(cfg.NC):
        fslice = np.zeros((F, cfg.ROWS), np.float32)
        fslice[:, :cfg.NPC] = feat[c * cfg.NPC:(c + 1) * cfg.NPC].T
        m = dict(
            featT=fslice,
            w1aug=w1a, w2aug=w2a,
            osrc=per_core_edges[c]["osrc"],
            odst=per_core_edges[c]["odst"],
            osrc2=per_core_edges[c]["osrc2"],
            odst2=per_core_edges[c]["odst2"],
            dloc=per_core_edges[c]["dloc"],
            iota=iota, b1rep=b1r, b2rep=b2r, sentel=sentel,
        )
        in_maps.append(m)
    return in_maps


_CACHE = {}


def _get_program(cfg, src, dst):
    per_core, perms, Tb, T, blk_of, k_of = prep_inputs(cfg, src, dst)
    key = (cfg.N, cfg.E, cfg.NC, tuple(Tb), tuple(blk_of), tuple(k_of))
    if key not in _CACHE:
        _CACHE[key] = build_nc(cfg, Tb, T, blk_of, k_of)
    return _CACHE[key], per_core, perms


def kernel(feat, src, dst, W1, al1, ar1, b1, W2, al2, ar2, b2,
           _trace=False, _return_results=False):
    from concourse.bass_utils import run_bass_kernel_spmd

    cfg = Cfg(100000, 800000, 8)
    nc, per_core, perms = _get_program(cfg, src, dst)
    in_maps = make_in_maps(cfg, per_core, feat, W1, al1, ar1, b1,
                           W2, al2, ar2, b2)
    res = run_bass_kernel_spmd(nc, in_maps, list(range(cfg.NC)), trace=_trace)
    out = np.zeros((cfg.N, OUT), np.float32)
    for c in range(cfg.NC):
        oc = np.asarray(res.results[c]["out"])  # [ROWS, 32], rows are relabeled
        out[c * cfg.NPC:(c + 1) * cfg.NPC] = oc[perms[c]]
    if _return_results:
        return out, res
    return out
